# revision 4
# baseline (speedup 1.0000x reference)
"""Trainium2 Bass kernel for nn_DeepLSTM: 2-layer LSTM (B=4096, T=1024, I=2, H=16, O=5).

V2 design (pure data parallel over batch, 8 cores x 512 batch each):
  - Lag pipeline: tick t computes cell1 step t, cell2 step t-1, linear step t-2.
    With lag-1 for cell2, both cells read the SAME h1(t-1) rows; with lag-2 for
    y, it reads the same h2(t-2) rows as cell2 -- so each hidden state has
    exactly ONE SBUF home and the per-tick h distribution is a single
    [32, 512] PSUM->SBUF copy.
  - One combined stationary per (tick, block): z = [h1(16) | h2(16) | 1 | x(t)
    | x(t-1) | x(t-2) | pad] (40 rows x 128 batch, bf16).  Two matmuls stream
    the static weight matrix columns over it: gates (N=128: [i f o g] x 2
    cells) and y (N=8).  4 LDW + 8 MM + 4 transposes per tick on PE.
  - Elementwise batch-major, merged across cells and blocks:
    sigmoid(ifo) + tanh(g) + mul + add + tanh(c) + mul = 6 ACT/DVE ops per
    tick; c-state kept in bf16 for DVE 2x modes.
  - x stream host-packed as [chunk][8 rows][16 ticks][512 batch] so each
    16-tick chunk is ONE dma_start with 8 fat fully-contiguous descriptors.
  - y staged 64 steps then written with [b][p][t][o]-major DRAM layout
    (2KB descriptor runs); final transpose done on host.
"""

import os
import sys

import numpy as np

sys.path.insert(0, "/opt/trn_rl_repo")
os.environ.setdefault("JAX_PLATFORMS", "")

import concourse.bass as bass
import concourse.bacc as bacc
import concourse.mybir as mybir
import concourse.tile as tile
from concourse.bass_utils import run_bass_kernel_spmd

import ml_dtypes

BF16 = mybir.dt.bfloat16
F32 = mybir.dt.float32
AF = mybir.ActivationFunctionType

B, T, I, H, O = 4096, 1024, 2, 16, 5
NCORES = 8
BC = B // NCORES      # 512 batch per core
NBLK = BC // 128      # 4 blocks of 128 batch
CHUNK = 16            # ticks per z chunk tile
YCH = 64              # y steps per staging tile / output DMA
KZ = 40               # z rows: h1 16 | h2 16 | 1 | x(t) 2 | x(t-1) 2 | x(t-2) 2 | pad
NPRE = 4              # chunks of x prefetch distance

# z row layout
R_H1, R_H2, R_ONE, R_XT, R_XT1, R_XT2, R_PAD = 0, 16, 32, 33, 35, 37, 39
# weight cols per block of gp: [i1 f1 o1 g1 | i2 f2 o2 g2]; y cols 128:136


def build_program(Tloc=T, trace_sim=False):
    nc = bacc.Bacc()
    nticks = Tloc + 2
    nch = (nticks + 1 + CHUNK - 1) // CHUNK  # cover copy at tick nticks-1 -> z(nticks)

    xq_d = nc.declare_dram_parameter("xq", [nch, 8, CHUNK * BC], BF16, isOutput=False)
    w_d = nc.declare_dram_parameter("w", [KZ, 136], BF16, isOutput=False)
    id_d = nc.declare_dram_parameter("ident", [128, 128], BF16, isOutput=False)
    y_d = nc.declare_dram_parameter("y", [NBLK, 128, Tloc, 8], F32, isOutput=True)

    with tile.TileContext(nc, trace_sim=trace_sim) as tc:
        with (
            tc.tile_pool(name="const", bufs=1) as cpool,
            tc.tile_pool(name="state", bufs=1) as spool,
            tc.tile_pool(name="z", bufs=6) as zpool,
            tc.tile_pool(name="scr", bufs=2) as scr,
            tc.tile_pool(name="ystage", bufs=2) as ypool,
            tc.tile_pool(name="gpsum", bufs=3, space="PSUM") as gp_pool,
            tc.tile_pool(name="hpsum", bufs=2, space="PSUM") as hp_pool,
        ):
            # ---- constants ----
            w_s = cpool.tile([KZ, 136], BF16, name="w_s")
            id_s = cpool.tile([128, 128], BF16, name="id_s")
            nc.sync.dma_start(w_s[:], w_d[:])
            nc.sync.dma_start(id_s[:], id_d[:])
            nc.tensor.ldweights(w_s[:, 0:1])
            nc.tensor.ldweights(id_s[:, 0:1])

            # ---- persistent state: [tg(16) | c(16)] per (blk, cell) ----
            st = spool.tile([128, NBLK * 2 * 32], BF16, name="st")
            stc = spool.tile([128, 1], F32, name="stc")
            nc.vector.memset(st[:], 0.0)
            nc.scalar.copy(stc[:], st[:, 0:1])  # ACT-side carrier for memset sem
            st4 = st.rearrange("p (b c s) -> p b c s", b=NBLK, c=2)

            # ---- z chunk tiles ----
            zs = [zpool.tile([KZ, CHUNK * BC], BF16, name=f"z{k}", tag="z") for k in range(nch)]
            # zero the h rows read before first writes (ticks 0 and 1)
            nc.vector.memset(zs[0][0:32, 0 : 2 * BC], 0.0)
            nc.tensor.ldweights(zs[0][0:32, 0:1])

            def x_dma(k):
                nc.sync.dma_start(zs[k][32:40, :], xq_d[k])
                nc.tensor.ldweights(zs[k][0:40, 0:1])

            for k in range(min(NPRE, nch)):
                x_dma(k)

            ystages = []
            nych = (Tloc + YCH - 1) // YCH
            for k in range(nych):
                ystages.append(
                    ypool.tile([128, YCH * NBLK * 8], F32, name=f"ys{k}", tag="ys")
                )

            # ---- PE warm-up preamble: ~4us of back-to-back matmuls so the
            # HAM clock gate un-throttles the array before the recurrence ----
            wk = gp_pool.tile([128, 1024], F32, name="gp", tag="gp")
            for j in range(16):
                nc.tensor.matmul(wk[0:128, 0:256], id_s[:], st[:])
            del wk

            for t in range(nticks):
                if t % CHUNK == 0 and t // CHUNK + NPRE < nch:
                    x_dma(t // CHUNK + NPRE)

                z = zs[t // CHUNK]
                s = t % CHUNK
                zcol = lambda b: z[0:KZ, s * BC + b * 128 : s * BC + (b + 1) * 128]

                # ---------- matmuls ----------
                # gp spans 2 PSUM banks; blocks at 256-col offsets so each
                # [*, 136] matmul output stays within one bank.
                gp = gp_pool.tile([128, 1024], F32, name="gp", tag="gp")
                gp4 = gp.rearrange("p (b c k) -> p b c k", b=NBLK, c=4)
                nw = 136 if t >= 2 else 128  # y cols only once valid
                for b in range(NBLK):
                    nc.tensor.matmul(
                        gp[:, b * 256 : b * 256 + nw], zcol(b), w_s[:, 0:nw]
                    )

                # ---------- elementwise ----------
                cs = slice(0, 1) if t == 0 else slice(0, 2)

                sg = scr.tile([128, NBLK * 2 * 48], BF16, name="sg", tag="sg")
                sg4 = sg.rearrange("p (b c s) -> p b c s", b=NBLK, c=2)
                prod = scr.tile([128, NBLK * 2 * 32], BF16, name="prod", tag="prod")
                pr4 = prod.rearrange("p (b c s) -> p b c s", b=NBLK, c=2)
                tcs = scr.tile([128, NBLK * 2 * 16], BF16, name="tcs", tag="tcs")
                tc4 = tcs.rearrange("p (b c s) -> p b c s", b=NBLK, c=2)
                hbuf = scr.tile([128, NBLK * 32], BF16, name="hbuf", tag="hbuf")
                hb4 = hbuf.rearrange("p (b c s) -> p b c s", b=NBLK, c=2)

                # sigmoid over [i, f] of active cells (spine)
                nc.scalar.activation(sg4[:, :, cs, 0:32], gp4[:, :, cs, 0:32], AF.Sigmoid)
                # tanh(g) -> st tg slot (spine)
                nc.scalar.activation(st4[:, :, cs, 0:16], gp4[:, :, cs, 48:64], AF.Tanh)
                # f*c needs only sigmoid(i,f): runs while tanh(g) is still on ACT
                nc.vector.tensor_mul(
                    pr4[:, :, cs, 16:32], sg4[:, :, cs, 16:32], st4[:, :, cs, 16:32]
                )
                # i*tg (waits tanh_g)
                nc.vector.tensor_mul(
                    pr4[:, :, cs, 0:16], sg4[:, :, cs, 0:16], st4[:, :, cs, 0:16]
                )
                # sigmoid(o) off-spine: overlaps the DVE mul/add
                nc.scalar.activation(sg4[:, :, cs, 32:48], gp4[:, :, cs, 32:48], AF.Sigmoid)
                # c' = i*tg + f*c  (into st c slot)
                nc.vector.tensor_add(
                    st4[:, :, cs, 16:32], pr4[:, :, cs, 0:16], pr4[:, :, cs, 16:32]
                )
                # tanh(c')
                nc.scalar.activation(tc4[:, :, cs], st4[:, :, cs, 16:32], AF.Tanh)
                # h = sig_o * tanh(c'), split in block-halves so the first
                # transposes start while the second half multiplies
                nc.vector.tensor_mul(
                    hb4[:, 0:2, cs], sg4[:, 0:2, cs, 32:48], tc4[:, 0:2, cs]
                )
                nc.vector.tensor_mul(
                    hb4[:, 2:4, cs], sg4[:, 2:4, cs, 32:48], tc4[:, 2:4, cs]
                )

                # ---------- h transpose + single copy ----------
                hT = hp_pool.tile([32, 512], BF16, name="hT", tag="hT")
                ncl = 16 if t == 0 else 32
                for b in range(NBLK):
                    nc.tensor.transpose(
                        hT[0:ncl, b * 128 : (b + 1) * 128],
                        hbuf[:, b * 32 : b * 32 + ncl],
                        id_s[:],
                    )
                zn = zs[(t + 1) // CHUNK]
                sn = (t + 1) % CHUNK
                # two half-copies: blocks 0-1 land first so their next-tick
                # matmuls start while blocks 2-3 are still copying
                nc.vector.tensor_copy(
                    zn[0:ncl, sn * BC : sn * BC + 256], hT[0:ncl, 0:256]
                )
                nc.vector.tensor_copy(
                    zn[0:ncl, sn * BC + 256 : (sn + 1) * BC], hT[0:ncl, 256:512]
                )

                # ---------- y evacuation ----------
                if t >= 2:
                    s3 = t - 2
                    ys = ystages[s3 // YCH]
                    ys4 = ys.rearrange("p (b ts o) -> p b ts o", b=NBLK, ts=YCH)
                    pos = s3 % YCH
                    nc.vector.tensor_copy(
                        ys4[:, :, pos : pos + 1, :], gp4[:, :, 2:3, 0:8]
                    )
                    if pos == YCH - 1 or s3 == Tloc - 1:
                        k = s3 // YCH
                        t0 = k * YCH
                        n = min(YCH, Tloc - t0)
                        src = ystages[k].rearrange(
                            "p (b ts o) -> p b ts o", b=NBLK, ts=YCH
                        )[:, :, 0:n, :]
                        dst = y_d[:, :, t0 : t0 + n, :].rearrange("b p ts o -> p b ts o")
                        nc.sync.dma_start(dst, src)

    return nc


_prog_cache = {}


def _get_program(Tloc):
    if Tloc not in _prog_cache:
        nc = build_program(Tloc)
        nc.finalize()
        _prog_cache[Tloc] = nc
    return _prog_cache[Tloc]


def _prep_weights(W_ih1, W_hh1, b_ih1, b_hh1, W_ih2, W_hh2, b_ih2, b_hh2, W_l, b_l):
    """Build the combined [KZ, 136] weight/bias matrix (f32; cast later)."""
    w = np.zeros((KZ, 136), np.float32)
    # torch gate order rows: [i, f, g, o] x16 ; our col order per cell: [i f o g]
    perm = np.r_[0:16, 16:32, 48:64, 32:48]  # -> [i, f, o, g]

    def cell_cols(c):
        return slice(c * 64, (c + 1) * 64)

    # cell 1 (cols 0:64)
    w[R_H1:R_H1 + 16, 0:64] = W_hh1.T[:, perm]
    w[R_ONE, 0:64] = (b_ih1 + b_hh1)[perm]
    w[R_XT:R_XT + 2, 0:64] = W_ih1.T[:, perm]
    # cell 2 (cols 64:128); x2 = [x, h1]
    w[R_H1:R_H1 + 16, 64:128] = W_ih2.T[2:18][:, perm]
    w[R_H2:R_H2 + 16, 64:128] = W_hh2.T[:, perm]
    w[R_ONE, 64:128] = (b_ih2 + b_hh2)[perm]
    w[R_XT1:R_XT1 + 2, 64:128] = W_ih2.T[0:2][:, perm]
    # y (cols 128:136); x3 = [x, h2]
    w[R_H2:R_H2 + 16, 128:133] = W_l.T[2:18]
    w[R_ONE, 128:133] = b_l
    w[R_XT2:R_XT2 + 2, 128:133] = W_l.T[0:2]
    return w


def _prep_inputs(x, W_ih1, W_hh1, b_ih1, b_hh1, W_ih2, W_hh2, b_ih2, b_hh2, W_l, b_l):
    bf = ml_dtypes.bfloat16
    Tloc = x.shape[1]
    nticks = Tloc + 2
    nch = (nticks + 1 + CHUNK - 1) // CHUNK
    w = _prep_weights(
        W_ih1, W_hh1, b_ih1, b_hh1, W_ih2, W_hh2, b_ih2, b_hh2, W_l, b_l
    ).astype(bf)
    ident = np.eye(128, dtype=np.float32).astype(bf)

    maps = []
    for c in range(NCORES):
        xc = np.ascontiguousarray(x[c * BC : (c + 1) * BC])  # [BC, T, 2]
        xt = np.zeros((nch * CHUNK, 2, BC), np.float32)
        xt[:Tloc] = xc.transpose(1, 2, 0)
        # xq rows: [ones | x(t) | x(t-1) | x(t-2) | zero]
        xq = np.zeros((nch * CHUNK, 8, BC), np.float32)
        xq[:, 0] = 1.0
        xq[:, 1:3] = xt
        xq[1:, 3:5] = xt[:-1]
        xq[2:, 5:7] = xt[:-2]
        xqb = (
            xq.astype(bf)
            .reshape(nch, CHUNK, 8, BC)
            .transpose(0, 2, 1, 3)  # [nch, 8, CHUNK, BC]
            .reshape(nch, 8, CHUNK * BC)
        )
        maps.append(dict(xq=np.ascontiguousarray(xqb), w=w, ident=ident))
    return maps


def _assemble(results, Tloc):
    y = np.empty((B, Tloc, O), np.float32)
    for c in range(NCORES):
        yc = results[c]["y"]  # [NBLK, 128, Tloc, 8]
        y[c * BC : (c + 1) * BC] = yc.reshape(BC, Tloc, 8)[:, :, :O]
    return y


def run(inputs, trace=False, **kw):
    x = np.asarray(inputs["x"])
    Tloc = x.shape[1]
    nc = _get_program(Tloc)
    in_maps = _prep_inputs(**{k: np.asarray(v) for k, v in inputs.items()})
    res = run_bass_kernel_spmd(nc, in_maps, list(range(NCORES)), trace=trace, **kw)
    return _assemble(res.results, Tloc), res


def kernel(**inputs):
    y, _ = run(inputs)
    return y
